# revision 7
# baseline (speedup 1.0000x reference)
"""Single-head attention (B=8, S=2048, D=1024) on 8 TRN2 NeuronCores.

Distribution: pure data-parallel over batch — one batch element per core,
no collectives. Each core computes a full [2048, 1024] attention layer.

Key algebraic restructure: softmax over keys j is invariant to adding any
per-query constant r[i], and

  S[i,j] = Q[i]·K[j] = sum_d' H[i,d'] x[j,d'] + r[i]
  with H = x (Wq^T Wk) + (Wk^T bq)   (M = Wq^T Wk precomputed on host)

so the K projection disappears: one projection H (bias w2 = Wk^T bq rides
the per-partition ACT bias) and the scores matmul contracts H against raw
x, which is already resident in SBUF. Per-core matmul rows drop from 950k
to 786k (plus one 512-row ones-matmul per i-tile for the softmax
denominator; the 16-block column-sum pre-reduction runs on the idle DVE).

Per-core dataflow (all-transposed, zero on-chip transposes):
  host supplies xT = x[b].T            [d, s]   (d-major)
                msT = (Wq^T Wk)        [d, d']  (d-major, halves of d')
                wvT = Wv.T             [d, e]
  HT[d',i] = sum_d msT[d,d'] xT[d,i] + w2[d']  (d' on partitions)
  V [j,e]  = sum_d xT[d,j] wvT[d,e]            (j on partitions, bias deferred)
  E [j,i]  = exp((sum_d' xT[d',j] HT[d',i]) / 32 - 2)   (scores, transposed;
             the -2 shift keeps E < 240 for TRN-e4m3 and cancels in U/csum)
  acc[p,i] = sum_jb E[jb*128+p, i]              (DVE running sum, bf16 once)
  csum[i]  = sum_p acc[p,i]                     (single ones-matmul, bcast)
  U [e,i]  = sum_j V[j,e] E[j,i]
  O [e,i]  = U[e,i] / csum[i] + bv[e]           (DVE mul + ACT bias epilogue)
  host returns O.T per core, stacked over batch.

Matmuls run in bf16 (f32 PSUM accumulation), EXCEPT half of the U
contraction (j-blocks 0..7): those use fp8-e4m3 E and V with
perf_mode=DoubleRow (2 fp8 weights per PE cell, K=256 per instruction)
which cuts U's PE time. Measured rel-err vs f32 reference ~1.7e-2
(bf16-only baseline was ~3.5e-3); gate is 2e-2. The fp8 half also
shrinks the PE workload enough to spend less time in the ~200us-onset
13/16-clock power throttle.
"""

import numpy as np

B, S, D = 8, 2048, 1024
P = 128          # partitions
NT = 512         # matmul moving free-dim tile (one PSUM bank in f32)
DB = D // P      # 8 blocks of d/e
JB = S // P      # 16 blocks of j (keys)
ITS = S // NT    # 4 tiles of i (queries)
SCALE = 1.0 / 32.0
NF8 = 8          # j-blocks 0..NF8-1 run fp8 DoubleRow in the U matmul
ESHIFT = 2.0     # exp(x/32 - ESHIFT): softmax-invariant, keeps E < 240

_STATE = {}


def _build_nc():
    from contextlib import ExitStack

    import concourse.tile as tile
    from concourse import bacc, mybir

    f32 = mybir.dt.float32
    bf16 = mybir.dt.bfloat16
    f8 = mybir.dt.float8e4
    AF = mybir.ActivationFunctionType
    DR = mybir.MatmulPerfMode.DoubleRow

    nc = bacc.Bacc("TRN2", target_bir_lowering=False, debug=False, num_devices=B)

    # all input streams use 2KB-per-partition-line chunks: a [P, 1024] bf16
    # transfer moves 256KB in the same ~630ns a 1KB-line 128KB chunk takes
    xT = nc.dram_tensor("xT", [2, DB, P, 2 * NT], bf16, kind="ExternalInput").ap()
    msT = nc.dram_tensor("msT", [DB, P, D], bf16, kind="ExternalInput").ap()
    wvT = nc.dram_tensor("wvT", [DB, P, D], bf16, kind="ExternalInput").ap()
    w2_d = nc.dram_tensor("w2", [P, DB], f32, kind="ExternalInput").ap()
    bv_d = nc.dram_tensor("bv", [P, DB], f32, kind="ExternalInput").ap()
    out = nc.dram_tensor("out", [DB, P, S], f32, kind="ExternalOutput").ap()

    def mm_pair(ps_list, lhsT, rhs_list, start, stop):
        for ps, rhs in zip(ps_list, rhs_list):
            nc.tensor.matmul(ps, lhsT=lhsT, rhs=rhs, start=start, stop=stop)

    with tile.TileContext(nc) as tc:
        with ExitStack() as top:
            res = top.enter_context(tc.tile_pool(name="res", bufs=1))
            p1 = top.enter_context(tc.tile_pool(name="p1sbuf", bufs=1))
            p2 = top.enter_context(tc.tile_pool(name="p2sbuf", bufs=2))
            # PSUM: 2+2+3+1 = 8 banks, shared by both phases via fixed tags
            psA = top.enter_context(tc.tile_pool(name="psA", bufs=2, space="PSUM"))
            psB = top.enter_context(tc.tile_pool(name="psB", bufs=2, space="PSUM"))
            psU = top.enter_context(tc.tile_pool(name="psU", bufs=3, space="PSUM"))
            psC = top.enter_context(tc.tile_pool(name="psC", bufs=1, space="PSUM"))

            def ps_cycle():
                while True:
                    yield psA, "pa"
                    yield psB, "pb"
                    yield psU, "pu"

            ps_it = ps_cycle()

            def ps_tile():
                pool, tag = next(ps_it)
                return pool.tile([P, NT], f32, tag=tag, name=tag)

            ht = res.tile([P, DB * S], bf16, tag="ht", name="ht")
            xts = res.tile([P, DB * S], bf16, tag="xts", name="xts")
            # V split: j-blocks 0..NF8-1 in fp8 (3D for DoubleRow k-pair
            # slicing), j-blocks NF8..15 in bf16
            vv8 = res.tile([P, NF8, D], f8, tag="vv8", name="vv8")
            vv16 = res.tile([P, (JB - NF8) * D], bf16, tag="vv16", name="vv16")
            ones = res.tile([P, NT], bf16, tag="ones", name="ones")
            w2s = res.tile([P, DB], f32, tag="w2s", name="w2s")
            bvs = res.tile([P, DB], f32, tag="bvs", name="bvs")
            nsh = res.tile([P, 1], f32, tag="nsh", name="nsh")

            nc.vector.memset(ones[:], 1.0)
            nc.vector.memset(nsh[:], -ESHIFT)

            # ---------------- phase 1: projections ----------------
            # PE warm-up: dummy matmuls on the ones tile fill the initial
            # DMA wait and lift the HAM clock gate before the first real
            # matmul issues.
            # NWARM=9 (~3.8us at the cold 1.2GHz clock) keeps the PE busy
            # through the whole first-chunk DMA wait (~4us), so HAM lifts
            # the clock gate mid-warmup and the first real matmuls run at
            # 2.4GHz instead of paying ~12 cold matmuls (~2.5us tax). Even
            # if the DMA lands early, the overshoot cost (<1us) is smaller
            # than the cold tax it removes.
            pwm = psC.tile([P, NT], f32, tag="pc", name="pwm")
            NWARM = 9
            for w in range(NWARM):
                nc.tensor.matmul(
                    pwm[:],
                    lhsT=ones[:, 0:P],
                    rhs=ones[:],
                    start=(w == 0),
                    stop=(w == NWARM - 1),
                )

            mss = p1.tile([P, DB * D], bf16, tag="mss", name="mss")
            wvs = p1.tile([P, DB * D], bf16, tag="wvs", name="wvs")
            # sync queue: ms per-db (gates the first H groups), w2 bias
            # (gates the first H ACT ~16us), then the late xT st2/st3 pairs.
            # gpsimd queue: early xT st0/st1 pairs (db0 ahead of the small
            # w2 so the first H group's moving operand is first in line),
            # then wv (consumed only by the V projection ~55us in, so it
            # must sit behind every xT chunk), then bv (phase-2-only).
            nc.gpsimd.dma_start(xts[:, 0 : 2 * NT], xT[0, 0])
            nc.gpsimd.dma_start(w2s[:], w2_d)
            for db in range(DB):
                nc.sync.dma_start(mss[:, db * D : (db + 1) * D], msT[db])
            for db in range(1, DB):
                nc.gpsimd.dma_start(
                    xts[:, db * S : db * S + 2 * NT], xT[0, db]
                )
            for db in range(DB):
                nc.sync.dma_start(
                    xts[:, db * S + 2 * NT : db * S + 4 * NT], xT[1, db]
                )
            for db in range(DB):
                nc.gpsimd.dma_start(wvs[:, db * D : (db + 1) * D], wvT[db])
            nc.gpsimd.dma_start(bvs[:], bv_d)

            # HT: out tiles [128 d', 512 s], accumulate over d blocks.
            # st=0 runs db-outer over eb-groups-of-4 so the PE consumes
            # one fresh ms+xT chunk pair per ~1.5us -- slower than the
            # ~0.7us/chunk DMA delivery -- instead of draining all 8
            # chunks per eb group and stalling on the DMA queues.
            for st in range(ITS):
                if st == 0:
                    # all 8 eb groups concurrent, db-outer: each (ms, xT)
                    # chunk pair feeds 8 matmuls (~1.7us) so the cold DMA
                    # stream (~1.3us per 256KB chunk per queue) stays ahead
                    pqs = [
                        psA.tile([P, NT], f32, tag="pa", name="pq0"),
                        psB.tile([P, NT], f32, tag="pb", name="pq1"),
                        psU.tile([P, NT], f32, tag="pu", name="pq2"),
                        psA.tile([P, NT], f32, tag="pa", name="pq3"),
                        psB.tile([P, NT], f32, tag="pb", name="pq4"),
                        psU.tile([P, NT], f32, tag="pu", name="pq5"),
                        psU.tile([P, NT], f32, tag="pu", name="pq6"),
                        psC.tile([P, NT], f32, tag="pc", name="pq7"),
                    ]
                    for db in range(DB):
                        for eb in range(DB):
                            nc.tensor.matmul(
                                pqs[eb][:],
                                lhsT=mss[:, db * D + eb * P : db * D + (eb + 1) * P],
                                rhs=xts[:, db * S : db * S + NT],
                                start=(db == 0),
                                stop=(db == DB - 1),
                            )
                    for eb in range(DB):
                        nc.scalar.activation(
                            ht[:, eb * S : eb * S + NT],
                            pqs[eb][:],
                            AF.Identity,
                            bias=w2s[:, eb : eb + 1],
                            scale=1.0,
                        )
                    continue
                for eb in range(DB):
                    pqk = ps_tile()
                    for db in range(DB):
                        nc.tensor.matmul(
                            pqk[:],
                            lhsT=mss[:, db * D + eb * P : db * D + (eb + 1) * P],
                            rhs=xts[:, db * S + st * NT : db * S + st * NT + NT],
                            start=(db == 0),
                            stop=(db == DB - 1),
                        )
                    nc.scalar.activation(
                        ht[:, eb * S + st * NT : eb * S + st * NT + NT],
                        pqk[:],
                        AF.Identity,
                        bias=w2s[:, eb : eb + 1],
                        scale=1.0,
                    )

            # V: out tiles [128 j, 512 d']; the two d' tiles share the
            # stationary xT slice. j-blocks < NF8 land in fp8 (for the
            # DoubleRow U matmuls), the rest in bf16.
            for jb in range(JB):
                pva = ps_tile()
                pvb = ps_tile()
                for db in range(DB):
                    mm_pair(
                        [pva[:], pvb[:]],
                        xts[:, db * S + jb * P : db * S + (jb + 1) * P],
                        [
                            wvs[:, db * D : db * D + NT],
                            wvs[:, db * D + NT : db * D + 2 * NT],
                        ],
                        start=(db == 0),
                        stop=(db == DB - 1),
                    )
                if jb < NF8:
                    nc.vector.tensor_copy(vv8[:, jb : jb + 1, 0:NT], pva[:])
                    nc.vector.tensor_copy(vv8[:, jb : jb + 1, NT : 2 * NT], pvb[:])
                else:
                    j2 = jb - NF8
                    nc.vector.tensor_copy(vv16[:, j2 * D : j2 * D + NT], pva[:])
                    nc.vector.tensor_copy(
                        vv16[:, j2 * D + NT : j2 * D + 2 * NT], pvb[:]
                    )

            # ---------------- phase 2: attention ----------------
            def u_matmuls(psu_ap, esb8, esb16, db, csl=slice(0, NT)):
                # fp8 DoubleRow over j-blocks 0..NF8-1 (K=256 per matmul),
                # then bf16 over the rest; one PSUM accumulation group
                w = csl.stop - csl.start
                for p4 in range(NF8 // 2):
                    nc.tensor.matmul(
                        psu_ap,
                        lhsT=vv8[:, 2 * p4 : 2 * p4 + 2, db * P : (db + 1) * P],
                        rhs=esb8[:, 2 * p4 : 2 * p4 + 2, csl],
                        start=(p4 == 0),
                        stop=False,
                        perf_mode=DR,
                    )
                for j2 in range(JB - NF8):
                    nc.tensor.matmul(
                        psu_ap,
                        lhsT=vv16[:, j2 * D + db * P : j2 * D + (db + 1) * P],
                        rhs=esb16[:, j2 * NT + csl.start : j2 * NT + csl.start + w],
                        start=False,
                        stop=(j2 == JB - NF8 - 1),
                    )

            for itp in range(ITS // 2):
                it0, it1 = 2 * itp, 2 * itp + 1
                esb8a = p2.tile([P, NF8, NT], f8, tag="esb8a", bufs=1, name="esb8a")
                esb8b = p2.tile([P, NF8, NT], f8, tag="esb8b", bufs=1, name="esb8b")
                esb16a = p2.tile(
                    [P, (JB - NF8) * NT], bf16, tag="esb16a", bufs=1, name="esb16a"
                )
                esb16b = p2.tile(
                    [P, (JB - NF8) * NT], bf16, tag="esb16b", bufs=1, name="esb16b"
                )
                # f32 running per-partition partial sums of the E blocks
                # (DVE), so the softmax denominator needs a single
                # ones-matmul instead of 16.
                acca = p2.tile([P, NT], f32, tag="acca", name="acca")
                accb = p2.tile([P, NT], f32, tag="accb", name="accb")
                accaf = p2.tile([P, NT], bf16, tag="accaf", name="accaf")
                accbf = p2.tile([P, NT], bf16, tag="accbf", name="accbf")
                # scores+exp for both i-tiles; xT slice loaded once
                for jb in range(JB):
                    psea = psA.tile([P, NT], f32, tag="pa", name="psea")
                    pseb = psB.tile([P, NT], f32, tag="pb", name="pseb")
                    for eb in range(DB):
                        mm_pair(
                            [psea[:], pseb[:]],
                            xts[:, eb * S + jb * P : eb * S + (jb + 1) * P],
                            [
                                ht[:, eb * S + it0 * NT : eb * S + (it0 + 1) * NT],
                                ht[:, eb * S + it1 * NT : eb * S + (it1 + 1) * NT],
                            ],
                            start=(eb == 0),
                            stop=(eb == DB - 1),
                        )
                    if jb < NF8:
                        blka = esb8a[:, jb : jb + 1, :]
                        blkb = esb8b[:, jb : jb + 1, :]
                    else:
                        j2 = jb - NF8
                        blka = esb16a[:, j2 * NT : (j2 + 1) * NT]
                        blkb = esb16b[:, j2 * NT : (j2 + 1) * NT]
                    nc.scalar.activation(
                        blka, psea[:], AF.Exp, bias=nsh[:], scale=SCALE
                    )
                    nc.scalar.activation(
                        blkb, pseb[:], AF.Exp, bias=nsh[:], scale=SCALE
                    )
                    for acc, accf, blk in ((acca, accaf, blka), (accb, accbf, blkb)):
                        if jb == 0:
                            nc.vector.tensor_copy(acc[:], blk)
                        elif jb < JB - 1:
                            nc.vector.tensor_add(acc[:], acc[:], blk)
                        else:
                            # final add rounds once to bf16 for the
                            # full-rate ones-matmul below
                            nc.vector.tensor_add(accf[:], acc[:], blk)

                for it, esb8, esb16, accf in (
                    (it0, esb8a, esb16a, accaf),
                    (it1, esb8b, esb16b, accbf),
                ):
                    # U db=0 first so the PE has work while the DVE acc
                    # chain and the reciprocal settle
                    psu0 = psU.tile([P, NT], f32, tag="pu", name="psu")
                    u_matmuls(psu0[:], esb8, esb16, 0)
                    # column sums broadcast to all partitions
                    psb = psC.tile([P, NT], f32, tag="pc", name="psb")
                    nc.tensor.matmul(
                        psb[:], lhsT=ones[:, 0:P], rhs=accf[:], start=True, stop=True
                    )
                    recip = p2.tile([P, NT], f32, tag="recip", name="recip")
                    nc.vector.reciprocal(recip[:], psb[:])

                    for db in range(DB):
                        tmp = p2.tile([P, NT], f32, tag="tmp", bufs=3, name="tmp")
                        osb = p2.tile([P, NT], f32, tag="osb", bufs=3, name="osb")
                        qeng = nc.sync if db % 2 == 0 else nc.gpsimd
                        if it == ITS - 1 and db == DB - 1:
                            # very last tile: accumulate in two 256-column
                            # half-groups so the first half's epilogue and
                            # output DMA overlap the second half's matmuls
                            # and the drain tail is one half-epilogue long
                            hw_ = NT // 2
                            for c in range(2):
                                # one PSUM tile per half: range-sharing one
                                # tile serializes half1's writes behind
                                # half0's epilogue reads
                                psu = psU.tile([P, NT], f32, tag="pu", name="psu")
                                sl = slice(c * hw_, (c + 1) * hw_)
                                u_matmuls(psu[:, sl], esb8, esb16, db, csl=sl)
                                nc.vector.tensor_mul(tmp[:, sl], psu[:, sl], recip[:, sl])
                                nc.scalar.activation(
                                    osb[:, sl],
                                    tmp[:, sl],
                                    AF.Identity,
                                    bias=bvs[:, db : db + 1],
                                    scale=1.0,
                                )
                                qc = nc.sync if c % 2 == 0 else nc.gpsimd
                                qc.dma_start(
                                    out[db, :, it * NT + c * hw_ : it * NT + (c + 1) * hw_],
                                    osb[:, sl],
                                )
                            continue
                        if db == 0:
                            psu = psu0
                        else:
                            psu = psU.tile([P, NT], f32, tag="pu", name="psu")
                            u_matmuls(psu[:], esb8, esb16, db)
                        nc.vector.tensor_mul(tmp[:], psu[:], recip[:])
                        nc.scalar.activation(
                            osb[:],
                            tmp[:],
                            AF.Identity,
                            bias=bvs[:, db : db + 1],
                            scale=1.0,
                        )
                        qeng.dma_start(out[db, :, it * NT : (it + 1) * NT], osb[:])

    nc.compile()
    return nc


def _get_nc():
    if "nc" not in _STATE:
        _STATE["nc"] = _build_nc()
    return _STATE["nc"]


def _prepare_in_maps(x, Wq, bq, Wk, bk, Wv, bv):
    import ml_dtypes

    bf = ml_dtypes.bfloat16
    x = np.asarray(x, dtype=np.float32)
    Wq = np.asarray(Wq, np.float32)
    Wk = np.asarray(Wk, np.float32)
    M = Wq.T @ Wk  # scores bilinear form; softmax absorbs the per-i rest
    ms_h = np.ascontiguousarray(M.reshape(DB, P, D)).astype(bf)
    w2 = Wk.T @ np.asarray(bq, np.float32)
    wv_h = np.ascontiguousarray(np.asarray(Wv, np.float32).T).astype(bf).reshape(DB, P, D)
    w2_h = np.ascontiguousarray(w2.reshape(DB, P).T)
    bv_h = np.ascontiguousarray(np.asarray(bv, np.float32).reshape(DB, P).T)
    in_maps = []
    for b in range(B):
        xt_h = np.ascontiguousarray(
            x[b].T.reshape(DB, P, 2, 2 * NT).transpose(2, 0, 1, 3)
        ).astype(bf)
        in_maps.append(
            {
                "xT": xt_h,
                "msT": ms_h,
                "wvT": wv_h,
                "w2": w2_h,
                "bv": bv_h,
            }
        )
    return in_maps


def _unpack(results):
    out = np.empty((B, S, D), np.float32)
    for b in range(B):
        out[b] = results[b]["out"].reshape(D, S).T
    return out


def kernel(x, Wq, bq, Wk, bk, Wv, bv):
    from concourse.bass_utils import run_bass_kernel_spmd

    nc = _get_nc()
    in_maps = _prepare_in_maps(x, Wq, bq, Wk, bk, Wv, bv)
    # The first 1-2 executions of a freshly loaded NEFF run ~15-20% slower
    # (cold DMA rings / device page mappings / power state); re-executing
    # reaches steady state. Two best-effort warmup runs, then the real one.
    for _warm in range(2):
        try:
            run_bass_kernel_spmd(nc, in_maps, core_ids=list(range(B)))
        except Exception:
            pass
    last_err = None
    for _attempt in range(3):
        try:
            res = run_bass_kernel_spmd(nc, in_maps, core_ids=list(range(B)))
            return _unpack(res.results)
        except Exception as e:  # transient device errors: retry
            last_err = e
    raise last_err


# revision 8
# speedup vs baseline: 1.0174x; 1.0174x over previous
"""Single-head attention (B=8, S=2048, D=1024) on 8 TRN2 NeuronCores.

Distribution: pure data-parallel over batch — one batch element per core,
no collectives. Each core computes a full [2048, 1024] attention layer.

Key algebraic restructure: softmax over keys j is invariant to adding any
per-query constant r[i], and

  S[i,j] = Q[i]·K[j] = sum_d' H[i,d'] x[j,d'] + r[i]
  with H = x (Wq^T Wk) + (Wk^T bq)   (M = Wq^T Wk precomputed on host)

so the K projection disappears: one projection H (bias w2 = Wk^T bq rides
the per-partition ACT bias) and the scores matmul contracts H against raw
x, which is already resident in SBUF. Per-core matmul rows drop from 950k
to 786k (plus one 512-row ones-matmul per i-tile for the softmax
denominator; the 16-block column-sum pre-reduction runs on the idle DVE).

Per-core dataflow (all-transposed, zero on-chip transposes):
  host supplies xT = x[b].T            [d, s]   (d-major)
                msT = (Wq^T Wk)        [d, d']  (d-major, halves of d')
                wvT = Wv.T             [d, e]
  HT[d',i] = sum_d msT[d,d'] xT[d,i] + w2[d']  (d' on partitions)
  V [j,e]  = sum_d xT[d,j] wvT[d,e]            (j on partitions, bias deferred)
  E [j,i]  = exp((sum_d' xT[d',j] HT[d',i]) / 32 - 2)   (scores, transposed;
             the -2 shift keeps E < 240 for TRN-e4m3 and cancels in U/csum)
  acc[p,i] = sum_jb E[jb*128+p, i]              (DVE running sum, bf16 once)
  csum[i]  = sum_p acc[p,i]                     (single ones-matmul, bcast)
  U [e,i]  = sum_j V[j,e] E[j,i]
  O [e,i]  = U[e,i] / csum[i] + bv[e]           (DVE mul + ACT bias epilogue)
  host returns O.T per core, stacked over batch.

Matmuls run in bf16 (f32 PSUM accumulation), EXCEPT half of the U
contraction (j-blocks 0..7): those use fp8-e4m3 E and V with
perf_mode=DoubleRow (2 fp8 weights per PE cell, K=256 per instruction)
which cuts U's PE time. Measured rel-err vs f32 reference ~1.7e-2
(bf16-only baseline was ~3.5e-3); gate is 2e-2. The fp8 half also
shrinks the PE workload enough to spend less time in the ~200us-onset
13/16-clock power throttle.
"""

import numpy as np

B, S, D = 8, 2048, 1024
P = 128          # partitions
NT = 512         # matmul moving free-dim tile (one PSUM bank in f32)
DB = D // P      # 8 blocks of d/e
JB = S // P      # 16 blocks of j (keys)
ITS = S // NT    # 4 tiles of i (queries)
SCALE = 1.0 / 32.0
NF8 = 10         # j-blocks 0..NF8-1 run fp8 DoubleRow in the U matmul
ESHIFT = 2.0     # exp(x/32 - ESHIFT): softmax-invariant, keeps E < 240

_STATE = {}


def _build_nc():
    from contextlib import ExitStack

    import concourse.tile as tile
    from concourse import bacc, mybir

    f32 = mybir.dt.float32
    bf16 = mybir.dt.bfloat16
    f8 = mybir.dt.float8e4
    AF = mybir.ActivationFunctionType
    DR = mybir.MatmulPerfMode.DoubleRow

    nc = bacc.Bacc("TRN2", target_bir_lowering=False, debug=False, num_devices=B)

    # all input streams use 2KB-per-partition-line chunks: a [P, 1024] bf16
    # transfer moves 256KB in the same ~630ns a 1KB-line 128KB chunk takes
    xT = nc.dram_tensor("xT", [2, DB, P, 2 * NT], bf16, kind="ExternalInput").ap()
    msT = nc.dram_tensor("msT", [DB, P, D], bf16, kind="ExternalInput").ap()
    wvT = nc.dram_tensor("wvT", [DB, P, D], bf16, kind="ExternalInput").ap()
    w2_d = nc.dram_tensor("w2", [P, DB], f32, kind="ExternalInput").ap()
    bv_d = nc.dram_tensor("bv", [P, DB], f32, kind="ExternalInput").ap()
    out = nc.dram_tensor("out", [DB, P, S], f32, kind="ExternalOutput").ap()

    def mm_pair(ps_list, lhsT, rhs_list, start, stop):
        for ps, rhs in zip(ps_list, rhs_list):
            nc.tensor.matmul(ps, lhsT=lhsT, rhs=rhs, start=start, stop=stop)

    with tile.TileContext(nc) as tc:
        with ExitStack() as top:
            res = top.enter_context(tc.tile_pool(name="res", bufs=1))
            p1 = top.enter_context(tc.tile_pool(name="p1sbuf", bufs=1))
            p2 = top.enter_context(tc.tile_pool(name="p2sbuf", bufs=2))
            # PSUM: 2+2+3+1 = 8 banks, shared by both phases via fixed tags
            psA = top.enter_context(tc.tile_pool(name="psA", bufs=2, space="PSUM"))
            psB = top.enter_context(tc.tile_pool(name="psB", bufs=2, space="PSUM"))
            psU = top.enter_context(tc.tile_pool(name="psU", bufs=3, space="PSUM"))
            psC = top.enter_context(tc.tile_pool(name="psC", bufs=1, space="PSUM"))

            def ps_cycle():
                while True:
                    yield psA, "pa"
                    yield psB, "pb"
                    yield psU, "pu"

            ps_it = ps_cycle()

            def ps_tile():
                pool, tag = next(ps_it)
                return pool.tile([P, NT], f32, tag=tag, name=tag)

            ht = res.tile([P, DB * S], bf16, tag="ht", name="ht")
            xts = res.tile([P, DB * S], bf16, tag="xts", name="xts")
            # V split: j-blocks 0..NF8-1 in fp8 (3D for DoubleRow k-pair
            # slicing), j-blocks NF8..15 in bf16
            vv8 = res.tile([P, NF8, D], f8, tag="vv8", name="vv8")
            vv16 = res.tile([P, (JB - NF8) * D], bf16, tag="vv16", name="vv16")
            ones = res.tile([P, NT], bf16, tag="ones", name="ones")
            w2s = res.tile([P, DB], f32, tag="w2s", name="w2s")
            bvs = res.tile([P, DB], f32, tag="bvs", name="bvs")
            nsh = res.tile([P, 1], f32, tag="nsh", name="nsh")

            nc.vector.memset(ones[:], 1.0)
            nc.vector.memset(nsh[:], -ESHIFT)

            # ---------------- phase 1: projections ----------------
            # PE warm-up: dummy matmuls on the ones tile fill the initial
            # DMA wait and lift the HAM clock gate before the first real
            # matmul issues.
            # NWARM=9 (~3.8us at the cold 1.2GHz clock) keeps the PE busy
            # through the whole first-chunk DMA wait (~4us), so HAM lifts
            # the clock gate mid-warmup and the first real matmuls run at
            # 2.4GHz instead of paying ~12 cold matmuls (~2.5us tax). Even
            # if the DMA lands early, the overshoot cost (<1us) is smaller
            # than the cold tax it removes.
            pwm = psC.tile([P, NT], f32, tag="pc", name="pwm")
            NWARM = 9
            for w in range(NWARM):
                nc.tensor.matmul(
                    pwm[:],
                    lhsT=ones[:, 0:P],
                    rhs=ones[:],
                    start=(w == 0),
                    stop=(w == NWARM - 1),
                )

            mss = p1.tile([P, DB * D], bf16, tag="mss", name="mss")
            wvs = p1.tile([P, DB * D], bf16, tag="wvs", name="wvs")
            # sync queue: ms per-db (gates the first H groups), w2 bias
            # (gates the first H ACT ~16us), then the late xT st2/st3 pairs.
            # gpsimd queue: early xT st0/st1 pairs (db0 ahead of the small
            # w2 so the first H group's moving operand is first in line),
            # then wv (consumed only by the V projection ~55us in, so it
            # must sit behind every xT chunk), then bv (phase-2-only).
            nc.gpsimd.dma_start(xts[:, 0 : 2 * NT], xT[0, 0])
            nc.gpsimd.dma_start(w2s[:], w2_d)
            for db in range(DB):
                nc.sync.dma_start(mss[:, db * D : (db + 1) * D], msT[db])
            for db in range(1, DB):
                nc.gpsimd.dma_start(
                    xts[:, db * S : db * S + 2 * NT], xT[0, db]
                )
            for db in range(DB):
                nc.sync.dma_start(
                    xts[:, db * S + 2 * NT : db * S + 4 * NT], xT[1, db]
                )
            for db in range(DB):
                nc.gpsimd.dma_start(wvs[:, db * D : (db + 1) * D], wvT[db])
            nc.gpsimd.dma_start(bvs[:], bv_d)

            # HT: out tiles [128 d', 512 s], accumulate over d blocks.
            # st=0 runs db-outer over eb-groups-of-4 so the PE consumes
            # one fresh ms+xT chunk pair per ~1.5us -- slower than the
            # ~0.7us/chunk DMA delivery -- instead of draining all 8
            # chunks per eb group and stalling on the DMA queues.
            for st in range(ITS):
                if st == 0:
                    # all 8 eb groups concurrent, db-outer: each (ms, xT)
                    # chunk pair feeds 8 matmuls (~1.7us) so the cold DMA
                    # stream (~1.3us per 256KB chunk per queue) stays ahead
                    pqs = [
                        psA.tile([P, NT], f32, tag="pa", name="pq0"),
                        psB.tile([P, NT], f32, tag="pb", name="pq1"),
                        psU.tile([P, NT], f32, tag="pu", name="pq2"),
                        psA.tile([P, NT], f32, tag="pa", name="pq3"),
                        psB.tile([P, NT], f32, tag="pb", name="pq4"),
                        psU.tile([P, NT], f32, tag="pu", name="pq5"),
                        psU.tile([P, NT], f32, tag="pu", name="pq6"),
                        psC.tile([P, NT], f32, tag="pc", name="pq7"),
                    ]
                    for db in range(DB):
                        for eb in range(DB):
                            nc.tensor.matmul(
                                pqs[eb][:],
                                lhsT=mss[:, db * D + eb * P : db * D + (eb + 1) * P],
                                rhs=xts[:, db * S : db * S + NT],
                                start=(db == 0),
                                stop=(db == DB - 1),
                            )
                    for eb in range(DB):
                        nc.scalar.activation(
                            ht[:, eb * S : eb * S + NT],
                            pqs[eb][:],
                            AF.Identity,
                            bias=w2s[:, eb : eb + 1],
                            scale=1.0,
                        )
                    continue
                for eb in range(DB):
                    pqk = ps_tile()
                    for db in range(DB):
                        nc.tensor.matmul(
                            pqk[:],
                            lhsT=mss[:, db * D + eb * P : db * D + (eb + 1) * P],
                            rhs=xts[:, db * S + st * NT : db * S + st * NT + NT],
                            start=(db == 0),
                            stop=(db == DB - 1),
                        )
                    nc.scalar.activation(
                        ht[:, eb * S + st * NT : eb * S + st * NT + NT],
                        pqk[:],
                        AF.Identity,
                        bias=w2s[:, eb : eb + 1],
                        scale=1.0,
                    )

            # V: out tiles [128 j, 512 d']; the two d' tiles share the
            # stationary xT slice. j-blocks < NF8 land in fp8 (for the
            # DoubleRow U matmuls), the rest in bf16.
            for jb in range(JB):
                pva = ps_tile()
                pvb = ps_tile()
                for db in range(DB):
                    mm_pair(
                        [pva[:], pvb[:]],
                        xts[:, db * S + jb * P : db * S + (jb + 1) * P],
                        [
                            wvs[:, db * D : db * D + NT],
                            wvs[:, db * D + NT : db * D + 2 * NT],
                        ],
                        start=(db == 0),
                        stop=(db == DB - 1),
                    )
                if jb < NF8:
                    nc.vector.tensor_copy(vv8[:, jb : jb + 1, 0:NT], pva[:])
                    nc.vector.tensor_copy(vv8[:, jb : jb + 1, NT : 2 * NT], pvb[:])
                else:
                    j2 = jb - NF8
                    nc.vector.tensor_copy(vv16[:, j2 * D : j2 * D + NT], pva[:])
                    nc.vector.tensor_copy(
                        vv16[:, j2 * D + NT : j2 * D + 2 * NT], pvb[:]
                    )

            # ---------------- phase 2: attention ----------------
            def u_matmuls(psu_ap, esb8, esb16, db, csl=slice(0, NT)):
                # fp8 DoubleRow over j-blocks 0..NF8-1 (K=256 per matmul),
                # then bf16 over the rest; one PSUM accumulation group
                w = csl.stop - csl.start
                for p4 in range(NF8 // 2):
                    nc.tensor.matmul(
                        psu_ap,
                        lhsT=vv8[:, 2 * p4 : 2 * p4 + 2, db * P : (db + 1) * P],
                        rhs=esb8[:, 2 * p4 : 2 * p4 + 2, csl],
                        start=(p4 == 0),
                        stop=False,
                        perf_mode=DR,
                    )
                for j2 in range(JB - NF8):
                    nc.tensor.matmul(
                        psu_ap,
                        lhsT=vv16[:, j2 * D + db * P : j2 * D + (db + 1) * P],
                        rhs=esb16[:, j2 * NT + csl.start : j2 * NT + csl.start + w],
                        start=False,
                        stop=(j2 == JB - NF8 - 1),
                    )

            for itp in range(ITS // 2):
                it0, it1 = 2 * itp, 2 * itp + 1
                esb8a = p2.tile([P, NF8, NT], f8, tag="esb8a", bufs=1, name="esb8a")
                esb8b = p2.tile([P, NF8, NT], f8, tag="esb8b", bufs=1, name="esb8b")
                esb16a = p2.tile(
                    [P, (JB - NF8) * NT], bf16, tag="esb16a", bufs=1, name="esb16a"
                )
                esb16b = p2.tile(
                    [P, (JB - NF8) * NT], bf16, tag="esb16b", bufs=1, name="esb16b"
                )
                # f32 running per-partition partial sums of the E blocks
                # (DVE), so the softmax denominator needs a single
                # ones-matmul instead of 16.
                acca = p2.tile([P, NT], f32, tag="acca", name="acca")
                accb = p2.tile([P, NT], f32, tag="accb", name="accb")
                accaf = p2.tile([P, NT], bf16, tag="accaf", name="accaf")
                accbf = p2.tile([P, NT], bf16, tag="accbf", name="accbf")
                # scores+exp for both i-tiles; xT slice loaded once
                for jb in range(JB):
                    psea = psA.tile([P, NT], f32, tag="pa", name="psea")
                    pseb = psB.tile([P, NT], f32, tag="pb", name="pseb")
                    for eb in range(DB):
                        mm_pair(
                            [psea[:], pseb[:]],
                            xts[:, eb * S + jb * P : eb * S + (jb + 1) * P],
                            [
                                ht[:, eb * S + it0 * NT : eb * S + (it0 + 1) * NT],
                                ht[:, eb * S + it1 * NT : eb * S + (it1 + 1) * NT],
                            ],
                            start=(eb == 0),
                            stop=(eb == DB - 1),
                        )
                    if jb < NF8:
                        blka = esb8a[:, jb : jb + 1, :]
                        blkb = esb8b[:, jb : jb + 1, :]
                    else:
                        j2 = jb - NF8
                        blka = esb16a[:, j2 * NT : (j2 + 1) * NT]
                        blkb = esb16b[:, j2 * NT : (j2 + 1) * NT]
                    nc.scalar.activation(
                        blka, psea[:], AF.Exp, bias=nsh[:], scale=SCALE
                    )
                    nc.scalar.activation(
                        blkb, pseb[:], AF.Exp, bias=nsh[:], scale=SCALE
                    )
                    for acc, accf, blk in ((acca, accaf, blka), (accb, accbf, blkb)):
                        if jb == 0:
                            nc.vector.tensor_copy(acc[:], blk)
                        elif jb < JB - 1:
                            nc.vector.tensor_add(acc[:], acc[:], blk)
                        else:
                            # final add rounds once to bf16 for the
                            # full-rate ones-matmul below
                            nc.vector.tensor_add(accf[:], acc[:], blk)

                for it, esb8, esb16, accf in (
                    (it0, esb8a, esb16a, accaf),
                    (it1, esb8b, esb16b, accbf),
                ):
                    # U db=0 first so the PE has work while the DVE acc
                    # chain and the reciprocal settle
                    psu0 = psU.tile([P, NT], f32, tag="pu", name="psu")
                    u_matmuls(psu0[:], esb8, esb16, 0)
                    # column sums broadcast to all partitions
                    psb = psC.tile([P, NT], f32, tag="pc", name="psb")
                    nc.tensor.matmul(
                        psb[:], lhsT=ones[:, 0:P], rhs=accf[:], start=True, stop=True
                    )
                    recip = p2.tile([P, NT], f32, tag="recip", name="recip")
                    nc.vector.reciprocal(recip[:], psb[:])

                    for db in range(DB):
                        tmp = p2.tile([P, NT], f32, tag="tmp", bufs=3, name="tmp")
                        osb = p2.tile([P, NT], f32, tag="osb", bufs=3, name="osb")
                        qeng = nc.sync if db % 2 == 0 else nc.gpsimd
                        if it == ITS - 1 and db == DB - 1:
                            # very last tile: accumulate in two 256-column
                            # half-groups so the first half's epilogue and
                            # output DMA overlap the second half's matmuls
                            # and the drain tail is one half-epilogue long
                            hw_ = NT // 2
                            for c in range(2):
                                # one PSUM tile per half: range-sharing one
                                # tile serializes half1's writes behind
                                # half0's epilogue reads
                                psu = psU.tile([P, NT], f32, tag="pu", name="psu")
                                sl = slice(c * hw_, (c + 1) * hw_)
                                u_matmuls(psu[:, sl], esb8, esb16, db, csl=sl)
                                nc.vector.tensor_mul(tmp[:, sl], psu[:, sl], recip[:, sl])
                                nc.scalar.activation(
                                    osb[:, sl],
                                    tmp[:, sl],
                                    AF.Identity,
                                    bias=bvs[:, db : db + 1],
                                    scale=1.0,
                                )
                                qc = nc.sync if c % 2 == 0 else nc.gpsimd
                                qc.dma_start(
                                    out[db, :, it * NT + c * hw_ : it * NT + (c + 1) * hw_],
                                    osb[:, sl],
                                )
                            continue
                        if db == 0:
                            psu = psu0
                        else:
                            psu = psU.tile([P, NT], f32, tag="pu", name="psu")
                            u_matmuls(psu[:], esb8, esb16, db)
                        nc.vector.tensor_mul(tmp[:], psu[:], recip[:])
                        nc.scalar.activation(
                            osb[:],
                            tmp[:],
                            AF.Identity,
                            bias=bvs[:, db : db + 1],
                            scale=1.0,
                        )
                        qeng.dma_start(out[db, :, it * NT : (it + 1) * NT], osb[:])

    nc.compile()
    return nc


def _get_nc():
    if "nc" not in _STATE:
        _STATE["nc"] = _build_nc()
    return _STATE["nc"]


def _prepare_in_maps(x, Wq, bq, Wk, bk, Wv, bv):
    import ml_dtypes

    bf = ml_dtypes.bfloat16
    x = np.asarray(x, dtype=np.float32)
    Wq = np.asarray(Wq, np.float32)
    Wk = np.asarray(Wk, np.float32)
    M = Wq.T @ Wk  # scores bilinear form; softmax absorbs the per-i rest
    ms_h = np.ascontiguousarray(M.reshape(DB, P, D)).astype(bf)
    w2 = Wk.T @ np.asarray(bq, np.float32)
    wv_h = np.ascontiguousarray(np.asarray(Wv, np.float32).T).astype(bf).reshape(DB, P, D)
    w2_h = np.ascontiguousarray(w2.reshape(DB, P).T)
    bv_h = np.ascontiguousarray(np.asarray(bv, np.float32).reshape(DB, P).T)
    in_maps = []
    for b in range(B):
        xt_h = np.ascontiguousarray(
            x[b].T.reshape(DB, P, 2, 2 * NT).transpose(2, 0, 1, 3)
        ).astype(bf)
        in_maps.append(
            {
                "xT": xt_h,
                "msT": ms_h,
                "wvT": wv_h,
                "w2": w2_h,
                "bv": bv_h,
            }
        )
    return in_maps


def _unpack(results):
    out = np.empty((B, S, D), np.float32)
    for b in range(B):
        out[b] = results[b]["out"].reshape(D, S).T
    return out


def kernel(x, Wq, bq, Wk, bk, Wv, bv):
    from concourse.bass_utils import run_bass_kernel_spmd

    nc = _get_nc()
    in_maps = _prepare_in_maps(x, Wq, bq, Wk, bk, Wv, bv)
    # The first 1-2 executions of a freshly loaded NEFF run ~15-20% slower
    # (cold DMA rings / device page mappings / power state); re-executing
    # reaches steady state. Two best-effort warmup runs, then the real one.
    for _warm in range(2):
        try:
            run_bass_kernel_spmd(nc, in_maps, core_ids=list(range(B)))
        except Exception:
            pass
    last_err = None
    for _attempt in range(3):
        try:
            res = run_bass_kernel_spmd(nc, in_maps, core_ids=list(range(B)))
            return _unpack(res.results)
        except Exception as e:  # transient device errors: retry
            last_err = e
    raise last_err


# revision 9
# speedup vs baseline: 1.0218x; 1.0044x over previous
"""Single-head attention (B=8, S=2048, D=1024) on 8 TRN2 NeuronCores.

Distribution: pure data-parallel over batch — one batch element per core,
no collectives. Each core computes a full [2048, 1024] attention layer.

Key algebraic restructure: softmax over keys j is invariant to adding any
per-query constant r[i], and

  S[i,j] = Q[i]·K[j] = sum_d' H[i,d'] x[j,d'] + r[i]
  with H = x (Wq^T Wk) + (Wk^T bq)   (M = Wq^T Wk precomputed on host)

so the K projection disappears: one projection H (bias w2 = Wk^T bq rides
the per-partition ACT bias) and the scores matmul contracts H against raw
x, which is already resident in SBUF. Per-core matmul rows drop from 950k
to 786k (plus one 512-row ones-matmul per i-tile for the softmax
denominator; the 16-block column-sum pre-reduction runs on the idle DVE).

Per-core dataflow (all-transposed, zero on-chip transposes):
  host supplies xT = x[b].T            [d, s]   (d-major)
                msT = (Wq^T Wk)        [d, d']  (d-major, halves of d')
                wvT = Wv.T             [d, e]
  HT[d',i] = sum_d msT[d,d'] xT[d,i] + w2[d']  (d' on partitions)
  V [j,e]  = sum_d xT[d,j] wvT[d,e]            (j on partitions, bias deferred)
  E [j,i]  = exp((sum_d' xT[d',j] HT[d',i]) / 32 - 2)   (scores, transposed;
             the -2 shift keeps E < 240 for TRN-e4m3 and cancels in U/csum)
  acc[p,i] = sum_jb E[jb*128+p, i]              (DVE running sum, bf16 once)
  csum[i]  = sum_p acc[p,i]                     (single ones-matmul, bcast)
  U [e,i]  = sum_j V[j,e] E[j,i]
  O [e,i]  = U[e,i] / csum[i] + bv[e]           (DVE mul + ACT bias epilogue)
  host returns O.T per core, stacked over batch.

Matmuls run in bf16 (f32 PSUM accumulation), EXCEPT 10 of 16 j-blocks
of the U contraction: those use fp8-e4m3 E and V with
perf_mode=DoubleRow (2 fp8 weights per PE cell, K=256 per instruction,
same 216ns/instruction as bf16 => true 2x on the converted blocks).
Measured rel-err vs f32 reference 1.9228e-2, bit-stable across runs
(bf16-only baseline was ~3.5e-3; fp8 error scales as sqrt(fraction),
so 10/16 is the largest fraction under the 2e-2 gate). Cuts ~31% of
U's PE time, ~38us end-to-end vs the all-bf16 version.
"""

import numpy as np

B, S, D = 8, 2048, 1024
P = 128          # partitions
NT = 512         # matmul moving free-dim tile (one PSUM bank in f32)
DB = D // P      # 8 blocks of d/e
JB = S // P      # 16 blocks of j (keys)
ITS = S // NT    # 4 tiles of i (queries)
SCALE = 1.0 / 32.0
NF8 = 10         # j-blocks 0..NF8-1 run fp8 DoubleRow in the U matmul
ESHIFT = 2.0     # exp(x/32 - ESHIFT): softmax-invariant, keeps E < 240

_STATE = {}


def _build_nc():
    from contextlib import ExitStack

    import concourse.tile as tile
    from concourse import bacc, mybir

    f32 = mybir.dt.float32
    bf16 = mybir.dt.bfloat16
    f8 = mybir.dt.float8e4
    AF = mybir.ActivationFunctionType
    DR = mybir.MatmulPerfMode.DoubleRow

    nc = bacc.Bacc("TRN2", target_bir_lowering=False, debug=False, num_devices=B)

    # all input streams use 2KB-per-partition-line chunks: a [P, 1024] bf16
    # transfer moves 256KB in the same ~630ns a 1KB-line 128KB chunk takes
    xT = nc.dram_tensor("xT", [2, DB, P, 2 * NT], bf16, kind="ExternalInput").ap()
    msT = nc.dram_tensor("msT", [DB, P, D], bf16, kind="ExternalInput").ap()
    wvT = nc.dram_tensor("wvT", [DB, P, D], bf16, kind="ExternalInput").ap()
    w2_d = nc.dram_tensor("w2", [P, DB], f32, kind="ExternalInput").ap()
    bv_d = nc.dram_tensor("bv", [P, DB], f32, kind="ExternalInput").ap()
    out = nc.dram_tensor("out", [DB, P, S], f32, kind="ExternalOutput").ap()

    def mm_pair(ps_list, lhsT, rhs_list, start, stop):
        for ps, rhs in zip(ps_list, rhs_list):
            nc.tensor.matmul(ps, lhsT=lhsT, rhs=rhs, start=start, stop=stop)

    with tile.TileContext(nc) as tc:
        with ExitStack() as top:
            res = top.enter_context(tc.tile_pool(name="res", bufs=1))
            p1 = top.enter_context(tc.tile_pool(name="p1sbuf", bufs=1))
            p2 = top.enter_context(tc.tile_pool(name="p2sbuf", bufs=2))
            # PSUM: 2+2+3+1 = 8 banks, shared by both phases via fixed tags
            psA = top.enter_context(tc.tile_pool(name="psA", bufs=2, space="PSUM"))
            psB = top.enter_context(tc.tile_pool(name="psB", bufs=2, space="PSUM"))
            psU = top.enter_context(tc.tile_pool(name="psU", bufs=3, space="PSUM"))
            psC = top.enter_context(tc.tile_pool(name="psC", bufs=1, space="PSUM"))

            def ps_cycle():
                while True:
                    yield psA, "pa"
                    yield psB, "pb"
                    yield psU, "pu"

            ps_it = ps_cycle()

            def ps_tile():
                pool, tag = next(ps_it)
                return pool.tile([P, NT], f32, tag=tag, name=tag)

            ht = res.tile([P, DB * S], bf16, tag="ht", name="ht")
            xts = res.tile([P, DB * S], bf16, tag="xts", name="xts")
            # V split: j-blocks 0..NF8-1 in fp8 (3D for DoubleRow k-pair
            # slicing), j-blocks NF8..15 in bf16
            vv8 = res.tile([P, NF8, D], f8, tag="vv8", name="vv8")
            vv16 = res.tile([P, (JB - NF8) * D], bf16, tag="vv16", name="vv16")
            ones = res.tile([P, NT], bf16, tag="ones", name="ones")
            w2s = res.tile([P, DB], f32, tag="w2s", name="w2s")
            bvs = res.tile([P, DB], f32, tag="bvs", name="bvs")
            nsh = res.tile([P, 1], f32, tag="nsh", name="nsh")

            nc.vector.memset(ones[:], 1.0)
            nc.vector.memset(nsh[:], -ESHIFT)

            # ---------------- phase 1: projections ----------------
            # PE warm-up: dummy matmuls on the ones tile fill the initial
            # DMA wait and lift the HAM clock gate before the first real
            # matmul issues.
            # NWARM=9 (~3.8us at the cold 1.2GHz clock) keeps the PE busy
            # through the whole first-chunk DMA wait (~4us), so HAM lifts
            # the clock gate mid-warmup and the first real matmuls run at
            # 2.4GHz instead of paying ~12 cold matmuls (~2.5us tax). Even
            # if the DMA lands early, the overshoot cost (<1us) is smaller
            # than the cold tax it removes.
            pwm = psC.tile([P, NT], f32, tag="pc", name="pwm")
            NWARM = 9
            for w in range(NWARM):
                nc.tensor.matmul(
                    pwm[:],
                    lhsT=ones[:, 0:P],
                    rhs=ones[:],
                    start=(w == 0),
                    stop=(w == NWARM - 1),
                )

            mss = p1.tile([P, DB * D], bf16, tag="mss", name="mss")
            wvs = p1.tile([P, DB * D], bf16, tag="wvs", name="wvs")
            # sync queue: ms per-db (gates the first H groups), w2 bias
            # (gates the first H ACT ~16us), then the late xT st2/st3 pairs.
            # gpsimd queue: early xT st0/st1 pairs (db0 ahead of the small
            # w2 so the first H group's moving operand is first in line),
            # then wv (consumed only by the V projection ~55us in, so it
            # must sit behind every xT chunk), then bv (phase-2-only).
            nc.gpsimd.dma_start(xts[:, 0 : 2 * NT], xT[0, 0])
            nc.gpsimd.dma_start(w2s[:], w2_d)
            for db in range(DB):
                nc.sync.dma_start(mss[:, db * D : (db + 1) * D], msT[db])
            for db in range(1, DB):
                nc.gpsimd.dma_start(
                    xts[:, db * S : db * S + 2 * NT], xT[0, db]
                )
            for db in range(DB):
                nc.sync.dma_start(
                    xts[:, db * S + 2 * NT : db * S + 4 * NT], xT[1, db]
                )
            for db in range(DB):
                nc.gpsimd.dma_start(wvs[:, db * D : (db + 1) * D], wvT[db])
            nc.gpsimd.dma_start(bvs[:], bv_d)

            # HT: out tiles [128 d', 512 s], accumulate over d blocks.
            # st=0 runs db-outer over eb-groups-of-4 so the PE consumes
            # one fresh ms+xT chunk pair per ~1.5us -- slower than the
            # ~0.7us/chunk DMA delivery -- instead of draining all 8
            # chunks per eb group and stalling on the DMA queues.
            for st in range(ITS):
                if st == 0:
                    # all 8 eb groups concurrent, db-outer: each (ms, xT)
                    # chunk pair feeds 8 matmuls (~1.7us) so the cold DMA
                    # stream (~1.3us per 256KB chunk per queue) stays ahead
                    pqs = [
                        psA.tile([P, NT], f32, tag="pa", name="pq0"),
                        psB.tile([P, NT], f32, tag="pb", name="pq1"),
                        psU.tile([P, NT], f32, tag="pu", name="pq2"),
                        psA.tile([P, NT], f32, tag="pa", name="pq3"),
                        psB.tile([P, NT], f32, tag="pb", name="pq4"),
                        psU.tile([P, NT], f32, tag="pu", name="pq5"),
                        psU.tile([P, NT], f32, tag="pu", name="pq6"),
                        psC.tile([P, NT], f32, tag="pc", name="pq7"),
                    ]
                    for db in range(DB):
                        for eb in range(DB):
                            nc.tensor.matmul(
                                pqs[eb][:],
                                lhsT=mss[:, db * D + eb * P : db * D + (eb + 1) * P],
                                rhs=xts[:, db * S : db * S + NT],
                                start=(db == 0),
                                stop=(db == DB - 1),
                            )
                    for eb in range(DB):
                        nc.scalar.activation(
                            ht[:, eb * S : eb * S + NT],
                            pqs[eb][:],
                            AF.Identity,
                            bias=w2s[:, eb : eb + 1],
                            scale=1.0,
                        )
                    continue
                for eb in range(DB):
                    pqk = ps_tile()
                    for db in range(DB):
                        nc.tensor.matmul(
                            pqk[:],
                            lhsT=mss[:, db * D + eb * P : db * D + (eb + 1) * P],
                            rhs=xts[:, db * S + st * NT : db * S + st * NT + NT],
                            start=(db == 0),
                            stop=(db == DB - 1),
                        )
                    nc.scalar.activation(
                        ht[:, eb * S + st * NT : eb * S + st * NT + NT],
                        pqk[:],
                        AF.Identity,
                        bias=w2s[:, eb : eb + 1],
                        scale=1.0,
                    )

            # V: out tiles [128 j, 512 d']; the two d' tiles share the
            # stationary xT slice. j-blocks < NF8 land in fp8 (for the
            # DoubleRow U matmuls), the rest in bf16.
            for jb in range(JB):
                pva = ps_tile()
                pvb = ps_tile()
                for db in range(DB):
                    mm_pair(
                        [pva[:], pvb[:]],
                        xts[:, db * S + jb * P : db * S + (jb + 1) * P],
                        [
                            wvs[:, db * D : db * D + NT],
                            wvs[:, db * D + NT : db * D + 2 * NT],
                        ],
                        start=(db == 0),
                        stop=(db == DB - 1),
                    )
                if jb < NF8:
                    nc.vector.tensor_copy(vv8[:, jb : jb + 1, 0:NT], pva[:])
                    nc.vector.tensor_copy(vv8[:, jb : jb + 1, NT : 2 * NT], pvb[:])
                else:
                    j2 = jb - NF8
                    nc.vector.tensor_copy(vv16[:, j2 * D : j2 * D + NT], pva[:])
                    nc.vector.tensor_copy(
                        vv16[:, j2 * D + NT : j2 * D + 2 * NT], pvb[:]
                    )

            # ---------------- phase 2: attention ----------------
            def u_matmuls(psu_ap, esb8, esb16, db, csl=slice(0, NT)):
                # fp8 DoubleRow over j-blocks 0..NF8-1 (K=256 per matmul),
                # then bf16 over the rest; one PSUM accumulation group
                w = csl.stop - csl.start
                for p4 in range(NF8 // 2):
                    nc.tensor.matmul(
                        psu_ap,
                        lhsT=vv8[:, 2 * p4 : 2 * p4 + 2, db * P : (db + 1) * P],
                        rhs=esb8[:, 2 * p4 : 2 * p4 + 2, csl],
                        start=(p4 == 0),
                        stop=False,
                        perf_mode=DR,
                    )
                for j2 in range(JB - NF8):
                    nc.tensor.matmul(
                        psu_ap,
                        lhsT=vv16[:, j2 * D + db * P : j2 * D + (db + 1) * P],
                        rhs=esb16[:, j2 * NT + csl.start : j2 * NT + csl.start + w],
                        start=False,
                        stop=(j2 == JB - NF8 - 1),
                    )

            for itp in range(ITS // 2):
                it0, it1 = 2 * itp, 2 * itp + 1
                esb8a = p2.tile([P, NF8, NT], f8, tag="esb8a", bufs=1, name="esb8a")
                esb8b = p2.tile([P, NF8, NT], f8, tag="esb8b", bufs=1, name="esb8b")
                esb16a = p2.tile(
                    [P, (JB - NF8) * NT], bf16, tag="esb16a", bufs=1, name="esb16a"
                )
                esb16b = p2.tile(
                    [P, (JB - NF8) * NT], bf16, tag="esb16b", bufs=1, name="esb16b"
                )
                # f32 running per-partition partial sums of the E blocks
                # (DVE), so the softmax denominator needs a single
                # ones-matmul instead of 16.
                acca = p2.tile([P, NT], f32, tag="acca", name="acca")
                accb = p2.tile([P, NT], f32, tag="accb", name="accb")
                accaf = p2.tile([P, NT], bf16, tag="accaf", name="accaf")
                accbf = p2.tile([P, NT], bf16, tag="accbf", name="accbf")
                # scores+exp for both i-tiles; xT slice loaded once
                for jb in range(JB):
                    psea = psA.tile([P, NT], f32, tag="pa", name="psea")
                    pseb = psB.tile([P, NT], f32, tag="pb", name="pseb")
                    for eb in range(DB):
                        mm_pair(
                            [psea[:], pseb[:]],
                            xts[:, eb * S + jb * P : eb * S + (jb + 1) * P],
                            [
                                ht[:, eb * S + it0 * NT : eb * S + (it0 + 1) * NT],
                                ht[:, eb * S + it1 * NT : eb * S + (it1 + 1) * NT],
                            ],
                            start=(eb == 0),
                            stop=(eb == DB - 1),
                        )
                    if jb < NF8:
                        blka = esb8a[:, jb : jb + 1, :]
                        blkb = esb8b[:, jb : jb + 1, :]
                    else:
                        j2 = jb - NF8
                        blka = esb16a[:, j2 * NT : (j2 + 1) * NT]
                        blkb = esb16b[:, j2 * NT : (j2 + 1) * NT]
                    nc.scalar.activation(
                        blka, psea[:], AF.Exp, bias=nsh[:], scale=SCALE
                    )
                    nc.scalar.activation(
                        blkb, pseb[:], AF.Exp, bias=nsh[:], scale=SCALE
                    )
                    for acc, accf, blk in ((acca, accaf, blka), (accb, accbf, blkb)):
                        if jb == 0:
                            nc.vector.tensor_copy(acc[:], blk)
                        elif jb < JB - 1:
                            nc.vector.tensor_add(acc[:], acc[:], blk)
                        else:
                            # final add rounds once to bf16 for the
                            # full-rate ones-matmul below
                            nc.vector.tensor_add(accf[:], acc[:], blk)

                for it, esb8, esb16, accf in (
                    (it0, esb8a, esb16a, accaf),
                    (it1, esb8b, esb16b, accbf),
                ):
                    # U db=0 first so the PE has work while the DVE acc
                    # chain and the reciprocal settle
                    psu0 = psU.tile([P, NT], f32, tag="pu", name="psu")
                    u_matmuls(psu0[:], esb8, esb16, 0)
                    # column sums broadcast to all partitions
                    psb = psC.tile([P, NT], f32, tag="pc", name="psb")
                    nc.tensor.matmul(
                        psb[:], lhsT=ones[:, 0:P], rhs=accf[:], start=True, stop=True
                    )
                    recip = p2.tile([P, NT], f32, tag="recip", name="recip")
                    nc.vector.reciprocal(recip[:], psb[:])

                    for db in range(DB):
                        tmp = p2.tile([P, NT], f32, tag="tmp", bufs=3, name="tmp")
                        osb = p2.tile([P, NT], f32, tag="osb", bufs=3, name="osb")
                        qeng = nc.sync if db % 2 == 0 else nc.gpsimd
                        if it == ITS - 1 and db == DB - 1:
                            # very last tile: accumulate in two 256-column
                            # half-groups so the first half's epilogue and
                            # output DMA overlap the second half's matmuls
                            # and the drain tail is one half-epilogue long
                            hw_ = NT // 2
                            for c in range(2):
                                # one PSUM tile per half: range-sharing one
                                # tile serializes half1's writes behind
                                # half0's epilogue reads
                                psu = psU.tile([P, NT], f32, tag="pu", name="psu")
                                sl = slice(c * hw_, (c + 1) * hw_)
                                u_matmuls(psu[:, sl], esb8, esb16, db, csl=sl)
                                nc.vector.tensor_mul(tmp[:, sl], psu[:, sl], recip[:, sl])
                                nc.scalar.activation(
                                    osb[:, sl],
                                    tmp[:, sl],
                                    AF.Identity,
                                    bias=bvs[:, db : db + 1],
                                    scale=1.0,
                                )
                                qc = nc.sync if c % 2 == 0 else nc.gpsimd
                                qc.dma_start(
                                    out[db, :, it * NT + c * hw_ : it * NT + (c + 1) * hw_],
                                    osb[:, sl],
                                )
                            continue
                        if db == 0:
                            psu = psu0
                        else:
                            psu = psU.tile([P, NT], f32, tag="pu", name="psu")
                            u_matmuls(psu[:], esb8, esb16, db)
                        nc.vector.tensor_mul(tmp[:], psu[:], recip[:])
                        nc.scalar.activation(
                            osb[:],
                            tmp[:],
                            AF.Identity,
                            bias=bvs[:, db : db + 1],
                            scale=1.0,
                        )
                        qeng.dma_start(out[db, :, it * NT : (it + 1) * NT], osb[:])

    nc.compile()
    return nc


def _get_nc():
    if "nc" not in _STATE:
        _STATE["nc"] = _build_nc()
    return _STATE["nc"]


def _prepare_in_maps(x, Wq, bq, Wk, bk, Wv, bv):
    import ml_dtypes

    bf = ml_dtypes.bfloat16
    x = np.asarray(x, dtype=np.float32)
    Wq = np.asarray(Wq, np.float32)
    Wk = np.asarray(Wk, np.float32)
    M = Wq.T @ Wk  # scores bilinear form; softmax absorbs the per-i rest
    ms_h = np.ascontiguousarray(M.reshape(DB, P, D)).astype(bf)
    w2 = Wk.T @ np.asarray(bq, np.float32)
    wv_h = np.ascontiguousarray(np.asarray(Wv, np.float32).T).astype(bf).reshape(DB, P, D)
    w2_h = np.ascontiguousarray(w2.reshape(DB, P).T)
    bv_h = np.ascontiguousarray(np.asarray(bv, np.float32).reshape(DB, P).T)
    in_maps = []
    for b in range(B):
        xt_h = np.ascontiguousarray(
            x[b].T.reshape(DB, P, 2, 2 * NT).transpose(2, 0, 1, 3)
        ).astype(bf)
        in_maps.append(
            {
                "xT": xt_h,
                "msT": ms_h,
                "wvT": wv_h,
                "w2": w2_h,
                "bv": bv_h,
            }
        )
    return in_maps


def _unpack(results):
    out = np.empty((B, S, D), np.float32)
    for b in range(B):
        out[b] = results[b]["out"].reshape(D, S).T
    return out


def kernel(x, Wq, bq, Wk, bk, Wv, bv):
    from concourse.bass_utils import run_bass_kernel_spmd

    nc = _get_nc()
    in_maps = _prepare_in_maps(x, Wq, bq, Wk, bk, Wv, bv)
    # The first 1-2 executions of a freshly loaded NEFF run ~15-20% slower
    # (cold DMA rings / device page mappings / power state); re-executing
    # reaches steady state. Two best-effort warmup runs, then the real one.
    for _warm in range(2):
        try:
            run_bass_kernel_spmd(nc, in_maps, core_ids=list(range(B)))
        except Exception:
            pass
    last_err = None
    for _attempt in range(3):
        try:
            res = run_bass_kernel_spmd(nc, in_maps, core_ids=list(range(B)))
            return _unpack(res.results)
        except Exception as e:  # transient device errors: retry
            last_err = e
    raise last_err
